# revision 32
# baseline (speedup 1.0000x reference)
"""Causal multi-head self-attention on 8 TRN2 NeuronCores — v4.

Sharding: batch (2) x head-group (4 heads = 256 contiguous features) -> 8
cores. Each core computes q/k/v projections for its 256 output features
from its batch's full activations, then causal attention for its 4 heads.
No collectives: the host concatenates the 8 [S, 256] shards.

Design (vs the fp32r/PE-transpose baseline at 230us):
  - bf16 internal precision (tolerance 2e-2; lands ~5e-3). All matmuls
    bf16 x bf16 -> f32 PSUM; LDWEIGHTS fits under the previous matmul's
    stream so the fp32r weight-shift stall is gone.
  - x and W are bf16-cast and transposed on the host during sharding, so
    xT/wT stream in as plain contiguous DMAs: no PE transposes, no
    PSUM->SBUF copies for operands. (DMA XBAR transposes were correct
    but cost ~1.25us each serialized on the sync queue.)
  - PV's stationary is padded to the full 128 PE columns
    ([v(64) | ones | zeros]): partial stationaries (M=80) measured
    ~1.5cyc/col vs 1.0 for full 128x128.
  - scores and PV of consecutive pairs interleave so the same PSUM bank
    is never written by back-to-back matmuls (RMW turnaround).
  - exp on Act engine processes kc-PAIRS ([128,1024] PSUM -> bf16 u):
    halves per-call overhead; Act is the second-busiest engine.
  - z normalization: zp row 64 = softmax row sums (ones column in V);
    zp -> zc bf16 -> 4 small bf16 PE transposes -> one reciprocal + 4
    per-qt muls; z stored bf16 and DMA'd out bf16 (host widens to f32).
  - startup: DMA issue costs ~0.65us each on a sequencer, so issues are
    split across the sync AND scalar HWDGE queues, constants are packed,
    and s-groups 1-3 stream as 3x-wide chunk DMAs (fewer issues, same
    ring bandwidth).
  - NOTE: DMAs sourced from f32r-declared DRAM tensors corrupt DGE
    descriptors (found empirically); DRAM tensors are f32/bf16 only.
"""

import sys

import ml_dtypes
import numpy as np

sys.path.insert(0, "/opt/trn_rl_repo")

import concourse.bass as bass
import concourse.tile as tile
from concourse import bacc, mybir
from concourse.bass_utils import run_bass_kernel_spmd

B, S, D, H = 2, 2048, 1024, 16
DK = D // H  # 64
NCORES = 8
HD = 256  # output features per core (4 heads x 64)
NHC = 4  # heads per core
NST = S // 128  # 16 s-tiles
NCC = D // 128  # 8 contraction chunks
NG = S // 512  # 4 query groups of 512
VW = 128  # v_aug stationary width: 64 v + 1 ones + 63 zeros (full PE width)
ZW = 80  # z-tail transpose width: 64 z + 1 sums + pad (16-multiple)

f32 = mybir.dt.float32
f32r = mybir.dt.float32r
bf16 = mybir.dt.bfloat16
AF = mybir.ActivationFunctionType
PSUM = bass.MemorySpace.PSUM


def _body(nc, tc, xt, wqt, wkt, wvt, consts, dmaskt, bv, out):
    with (
        tc.tile_pool(name="persist", bufs=1) as persist,
        tc.tile_pool(name="u", bufs=8) as u_pool,
        tc.tile_pool(name="zc", bufs=2) as zc_pool,
        tc.tile_pool(name="small", bufs=2) as small,
        tc.tile_pool(name="psum_sc", bufs=4, space=PSUM) as psum_sc,
        tc.tile_pool(name="psum_pr", bufs=2, space=PSUM) as psum_pr,
        tc.tile_pool(name="psum_z", bufs=2, space=PSUM) as psum_z,
    ):
        # consts: [0:128]=identity, [128:130]=bq cols, [130:132]=bk cols
        cst = persist.tile([128, 132], f32)
        nc.sync.dma_start(out=cst[:], in_=consts)
        # dmask[k, q] = 0.0 if q >= k else -8e9 (additive causal, diag block)
        dmask = persist.tile([128, 128], f32)
        nc.sync.dma_start(out=dmask[:], in_=dmaskt)
        bvf = persist.tile([1, HD], f32)
        nc.sync.dma_start(out=bvf[:], in_=bv[:])

        ident_bf = persist.tile([128, 128], bf16)
        nc.vector.tensor_copy(ident_bf[:], cst[:, 0:128])
        bv_sb = persist.tile([1, HD], f32r)
        nc.vector.tensor_copy(bv_sb[:], bvf[:])
        ones_row = persist.tile([1, 128], f32r)
        nc.scalar.activation(ones_row[:], cst[0:1, 0:128], AF.Copy, scale=0.0, bias=1.0)

        # ---- persistent operand tensors (all bf16) ----
        xT = persist.tile([128, NCC, S], bf16)  # 32KB/partition
        wqT = persist.tile([128, NCC, HD], bf16)
        wkT = persist.tile([128, NCC, HD], bf16)
        wvT = persist.tile([128, NCC, HD], bf16)
        qT = persist.tile([128, 2, S], bf16)
        # kTz: per-head K-padded k^T — head h occupies rows (h%2)*64..+64 of
        # its [128, S] slice, the other 64 rows are zero. Scores then run
        # with a full 128x128 stationary (1.0 cyc/col vs ~1.5 at K=64) and
        # the FULL qT (both heads) as moving operand: the zero rows kill
        # the other head's contribution.
        kTz = persist.tile([128, NHC, S], bf16)
        v_aug = persist.tile([128, NST, NHC, VW], bf16)
        z_full = persist.tile([128, NST, HD], bf16)
        nc.vector.memset(v_aug[:, :, :, 64], 1.0)
        nc.vector.memset(v_aug[:, :, :, 65:VW], 0.0)
        for h in range(NHC):
            po = (h % 2) * 64
            nc.vector.memset(kTz[64 - po : 128 - po, h, :], 0.0)

        # ---- input DMAs: issue split across sync + scalar HWDGE queues.
        # sg0/wq/wk/wv per-cc (parallel rings, early); sg1-3 as 3x-wide
        # chunks (3KB/partition rows -> same issue cost, 3x data).
        for cc in range(0, NCC, 2):
            nc.sync.dma_start(
                out=xT[:, cc, 0:512], in_=xt[bass.ts(cc, 128), 0:512]
            )
            nc.scalar.dma_start(
                out=xT[:, cc + 1, 0:512], in_=xt[bass.ts(cc + 1, 128), 0:512]
            )
        for cc in range(NCC):
            eng = nc.scalar if cc % 2 else nc.sync
            eng.dma_start(out=wqT[:, cc, :], in_=wqt[bass.ts(cc, 128), :])
        for cc in range(NCC):
            eng = nc.scalar if cc % 2 else nc.sync
            eng.dma_start(out=wkT[:, cc, :], in_=wkt[bass.ts(cc, 128), :])
        for cc in range(NCC):
            eng = nc.scalar if cc % 2 else nc.sync
            eng.dma_start(out=wvT[:, cc, :], in_=wvt[bass.ts(cc, 128), :])
        for cc in range(NCC):
            eng = nc.scalar if cc % 2 else nc.sync
            eng.dma_start(
                out=xT[:, cc, 512:S], in_=xt[bass.ts(cc, 128), 512:S]
            )

        # ---- projections for s-group sg (yields ~0.9us sub-units) ----
        def gen_qk(sg):
            # q/k: out [hd(128) x 512] per hdc bank, accumulate over 8 ccs
            for wT_t, bcol, dstT in ((wqT, 128, qT), (wkT, 130, None)):
                pa = psum_pr.tile([128, 512], f32, tag="pr", name="pa")
                pb = psum_pr.tile([128, 512], f32, tag="pr", name="pb")
                for cb in range(4):
                    for cc in (2 * cb, 2 * cb + 1):
                        for hdc, pp in ((0, pa), (1, pb)):
                            nc.tensor.matmul(
                                pp[:],
                                lhsT=wT_t[:, cc, bass.ts(hdc, 128)],
                                rhs=xT[:, cc, bass.ts(sg, 512)],
                                start=(cc == 0),
                                stop=(cc == NCC - 1),
                            )
                    if cb == 3:
                        for hdc, pp in ((0, pa), (1, pb)):
                            if dstT is not None:
                                nc.vector.tensor_scalar_add(
                                    dstT[:, hdc, bass.ts(sg, 512)],
                                    pp[:],
                                    cst[:, bcol + hdc : bcol + hdc + 1],
                                )
                            else:
                                # k: write each head's half into its padded
                                # kTz slice (other rows stay zero)
                                for hh in range(2):
                                    h = 2 * hdc + hh
                                    po = (h % 2) * 64
                                    nc.vector.tensor_scalar_add(
                                        kTz[po : po + 64, h, bass.ts(sg, 512)],
                                        pp[po : po + 64, :],
                                        cst[po : po + 64, bcol + hdc : bcol + hdc + 1],
                                    )
                    yield

        def gen_v(sg):
            # v: natural [s(128) x 256] per s-tile, pairs alternate banks
            for spair in range(2):
                pvs = [
                    psum_pr.tile([128, HD], f32, tag="pr", name=f"pv{i}")
                    for i in range(2)
                ]
                for cb in range(2):
                    for cc in range(4 * cb, 4 * cb + 4):
                        for stl in range(2):
                            nc.tensor.matmul(
                                pvs[stl][:],
                                lhsT=xT[:, cc, bass.ts(sg * 4 + spair * 2 + stl, 128)],
                                rhs=wvT[:, cc, :],
                                start=(cc == 0),
                                stop=False,
                            )
                    if cb == 1:
                        for stl in range(2):
                            st = sg * 4 + spair * 2 + stl
                            nc.tensor.matmul(
                                pvs[stl][:],
                                lhsT=ones_row[0:1, :],
                                rhs=bv_sb[0:1, :],
                                start=False,
                                stop=True,
                            )
                            nc.vector.tensor_copy(
                                v_aug[:, st, :, 0:64],
                                pvs[stl][:].rearrange("p (h d) -> p h d", h=NHC),
                            )
                    yield

        # ---- attention for query group g (512 queries) ----
        def gen_attn(g):
            nkc = 4 * g + 4
            for hp in (0, 2):
                chains = []
                for h in (hp, hp + 1):
                    zp = psum_z.tile([VW, 512], f32, tag="z", name=f"zp{h}")
                    chains.append({"h": h, "zp": zp, "prev": None})

                def flush_one(ch, t):
                    kc, u_p, q0 = ch["prev"][t]
                    nc.tensor.matmul(
                        ch["zp"][:, q0:512],
                        lhsT=v_aug[:, kc, ch["h"], :],
                        rhs=u_p[:, q0:512],
                        start=(kc == 0),
                        stop=(kc == nkc - 1),
                    )

                def do_stage(ch, kb):
                    # per-kc scores/exp with the prev pair's PV flushes
                    # interleaved so no PSUM bank sees back-to-back writes.
                    h = ch["h"]
                    hdc = h // 2
                    cur = []
                    for t, kc in enumerate((kb, kb + 1)):
                        j = kc - 4 * g
                        q0 = max(0, 128 * j)
                        sp = psum_sc.tile([128, 512], f32, tag="sc", name="sp")
                        nc.tensor.matmul(
                            sp[:, q0:512],
                            lhsT=kTz[:, h, bass.ts(kc, 128)],
                            rhs=qT[:, hdc, bass.ds(g * 512 + q0, 512 - q0)],
                            start=True,
                            stop=True,
                        )
                        if j >= 0:
                            nc.vector.tensor_add(
                                sp[:, q0 : q0 + 128], sp[:, q0 : q0 + 128], dmask[:]
                            )
                        u = u_pool.tile([128, 512], bf16, tag="u", name="u")
                        nc.scalar.activation(
                            u[:, q0:512], sp[:, q0:512], AF.Exp, scale=0.125
                        )
                        cur.append((kc, u, q0))
                        if ch["prev"] is not None:
                            flush_one(ch, t)
                    ch["prev"] = cur

                for kb in range(0, nkc, 2):
                    for ch in chains:
                        do_stage(ch, kb)
                    yield
                for ch in chains:
                    flush_one(ch, 0)
                    flush_one(ch, 1)
                    ch["prev"] = None

                # z tail: zp -> zc bf16 -> 4 bf16 PE transposes -> [q, ZW];
                # one reciprocal of the sums column, 4 per-qt muls. After the
                # last head's mul for a tile, its out DMA goes immediately.
                for ch in chains:
                    h = ch["h"]
                    zc = zc_pool.tile([ZW, 512], bf16, tag="zc", name="zc")
                    nc.vector.tensor_copy(zc[:], ch["zp"][0:ZW, :])
                    zt = psum_pr.tile([128, 4, ZW], bf16, tag="pr", name="zt")
                    for qt in range(4):
                        nc.tensor.transpose(
                            zt[:, qt, :],
                            zc[:, bass.ts(qt, 128)],
                            ident_bf[0:ZW, 0:ZW],
                        )
                    r4 = small.tile([128, 4], f32, tag="r", name="r4")
                    nc.vector.reciprocal(r4[:], zt[:, :, 64])
                    for qt in range(4):
                        nc.vector.tensor_scalar_mul(
                            z_full[:, g * 4 + qt, bass.ts(h, 64)],
                            zt[:, qt, 0:64],
                            r4[:, qt : qt + 1],
                        )
                        if h == 3:
                            st = g * 4 + qt
                            nc.sync.dma_start(
                                out=out[bass.ts(st, 128), :], in_=z_full[:, st, :]
                            )
                    yield
            yield

        def drain(gen):
            for _ in gen:
                pass

        # program-order interleave: attention for group g runs with the q/k
        # projections of group g+1 AND the v projection of group g itself
        # (v has no Act-engine work, so deferring it gives the Act-bound
        # attention phases — especially the g=3 tail — PE work to overlap).
        drain(gen_qk(0))
        drain(gen_v(0))
        for sg in range(NG):
            streams = [gen_attn(sg)]
            if sg >= 1:
                streams.append(gen_v(sg))
            if sg + 1 < NG:
                streams.append(gen_qk(sg + 1))
            while True:
                done = True
                for s in streams:
                    if next(s, StopIteration) is not StopIteration:
                        done = False
                if done:
                    break


def build():
    nc = bacc.Bacc(
        "TRN2", target_bir_lowering=False, debug=False, num_devices=NCORES
    )
    xt = nc.dram_tensor("xt", [D, S], bf16, kind="ExternalInput")
    wqt = nc.dram_tensor("wqt", [D, HD], bf16, kind="ExternalInput")
    wkt = nc.dram_tensor("wkt", [D, HD], bf16, kind="ExternalInput")
    wvt = nc.dram_tensor("wvt", [D, HD], bf16, kind="ExternalInput")
    consts = nc.dram_tensor("consts", [128, 132], f32, kind="ExternalInput")
    dmaskt = nc.dram_tensor("dmaskt", [128, 128], f32, kind="ExternalInput")
    bv = nc.dram_tensor("bv", [1, HD], f32, kind="ExternalInput")
    out = nc.dram_tensor("out", [S, HD], bf16, kind="ExternalOutput")
    with tile.TileContext(nc) as tc:
        _body(
            nc, tc, xt.ap(), wqt.ap(), wkt.ap(), wvt.ap(),
            consts.ap(), dmaskt.ap(), bv.ap(), out.ap(),
        )
    nc.compile()
    return nc


_NC_CACHE = None


def _get_nc():
    global _NC_CACHE
    if _NC_CACHE is None:
        _NC_CACHE = build()
    return _NC_CACHE


def make_in_maps(q_input, W_q, b_q, W_k, b_k, W_v, b_v):
    ii = np.arange(128)
    dmaskt = np.where(ii[None, :] >= ii[:, None], 0.0, -8.0e9).astype(np.float32)
    bf = ml_dtypes.bfloat16
    # host-side marshaling: bf16 cast + transpose (kernel-internal layout)
    xts = [np.ascontiguousarray(np.asarray(q_input[b]).T.astype(bf)) for b in range(B)]
    in_maps = []
    for c in range(NCORES):
        b = c // 4
        hs = slice((c % 4) * HD, (c % 4 + 1) * HD)
        consts = np.zeros((128, 132), np.float32)
        consts[:, 0:128] = np.eye(128, dtype=np.float32)
        consts[:, 128:130] = np.asarray(b_q[hs], dtype=np.float32).reshape(2, 128).T
        consts[:, 130:132] = np.asarray(b_k[hs], dtype=np.float32).reshape(2, 128).T
        in_maps.append(
            {
                "xt": xts[b],
                "wqt": np.ascontiguousarray(np.asarray(W_q[hs]).T.astype(bf)),
                "wkt": np.ascontiguousarray(np.asarray(W_k[hs]).T.astype(bf)),
                "wvt": np.ascontiguousarray(np.asarray(W_v[hs]).T.astype(bf)),
                "consts": consts,
                "dmaskt": dmaskt,
                "bv": np.ascontiguousarray(
                    np.asarray(b_v[hs], dtype=np.float32).reshape(1, HD)
                ),
            }
        )
    return in_maps


def assemble(results):
    full = np.empty((B, S, D), dtype=np.float32)
    for c in range(NCORES):
        b = c // 4
        hs = slice((c % 4) * HD, (c % 4 + 1) * HD)
        full[b, :, hs] = np.asarray(results[c]["out"]).astype(np.float32)
    return full


def _ensure_ntff_hook():
    """Register the axon NTFF profiling hook if the image's antenv lacks it."""
    try:
        from antenv import axon_hooks  # noqa: F401

        return
    except ImportError:
        pass
    import types

    try:
        from trn_agent_boot.trn_boot import _ntff_profile_via_ctypes

        hook = _ntff_profile_via_ctypes("/opt/axon/libaxon_pjrt.so")
    except Exception:
        hook = None
    mod = types.ModuleType("antenv.axon_hooks")
    mod._hook = hook
    mod.get_axon_ntff_profile_hook = lambda: mod._hook

    def _set(h):
        mod._hook = h

    mod.set_axon_ntff_profile_hook = _set
    sys.modules["antenv.axon_hooks"] = mod
    try:
        import antenv

        antenv.axon_hooks = mod
    except ImportError:
        pass


def run(inputs_dict, trace=False):
    """Run on hardware; returns (full_output, BassKernelResults)."""
    nc = _get_nc()
    if trace:
        _ensure_ntff_hook()
        import concourse.bass_utils as _bu

        _bu.upload_artifacts = lambda d: d  # no bucket access in this env
    in_maps = make_in_maps(**{k: np.asarray(v) for k, v in inputs_dict.items()})
    res = run_bass_kernel_spmd(nc, in_maps, core_ids=list(range(NCORES)), trace=trace)
    return assemble(res.results), res


def kernel(**inputs):
    out, _ = run(inputs, trace=False)
    return out
